# revision 1
# baseline (speedup 1.0000x reference)
"""Head-sharded causal self-attention (value-residual + RMSNorm + RoPE) for 8 TRN2 cores.

Sharding: 2 heads per core (tensor parallel). Each core computes q/k/v for its
128 dims, full causal attention for its heads, and a partial c_proj output;
the host sums the 8 partial [T, D] outputs (the TP all-reduce).

Layouts on device (per core):
  xT   [D=1024, T=2048]  (host-transposed)   q,k transposed [j', T]; v in [T, j'].
  Softmax without max-subtraction (RMS-normed q,k bound |scores| <= 8).
  Rowsum via a 64-wide ones block in the PV matmul lhsT -> denominator lands
  replicated on the opposite 64-partition half of the z PSUM tile.
  1/sqrt and 1/Z via exp(-a*ln(x)) on ScalarE (stays in one ACT table set).
"""
import os
import sys

sys.path.insert(0, "/opt/trn_rl_repo")

import numpy as np

import concourse.bacc as bacc
import concourse.tile as tile
import concourse.bass as bass
from concourse import mybir
from concourse.bass_utils import run_bass_kernel_spmd

N_CORES = 8
T, D, H, HD = 2048, 1024, 16, 64
HS = H // N_CORES            # 2 heads per core
J = HS * HD                  # 128
NT = T // 128                # 16 t-tiles
NCH = T // 512               # 4 chunks
KT = D // 128                # 8 contraction tiles
F32 = mybir.dt.float32
BF16 = mybir.dt.bfloat16
AF = mybir.ActivationFunctionType
OP = mybir.AluOpType
EPS = float(np.finfo(np.float32).eps)


def build_nc():
    nc = bacc.Bacc("TRN2", target_bir_lowering=False, debug=False,
                   num_devices=N_CORES)

    xT = nc.dram_tensor("xT", [D, T], F32, kind="ExternalInput")
    wqT = nc.dram_tensor("wqT", [D, J], F32, kind="ExternalInput")
    wkT = nc.dram_tensor("wkT", [D, J], F32, kind="ExternalInput")
    wvT = nc.dram_tensor("wvT", [D, J], F32, kind="ExternalInput")
    wpT = nc.dram_tensor("wpT", [J, D], F32, kind="ExternalInput")
    vic = nc.dram_tensor("vic", [T, J], F32, kind="ExternalInput")
    lam = nc.dram_tensor("lam", [2], F32, kind="ExternalInput")
    Ct = nc.dram_tensor("Ct", [J, T], F32, kind="ExternalInput")
    St = nc.dram_tensor("St", [J, T], F32, kind="ExternalInput")
    tri = nc.dram_tensor("tri", [128, 128], F32, kind="ExternalInput")
    o2r = nc.dram_tensor("o2r", [128, 128], F32, kind="ExternalInput")
    prm = nc.dram_tensor("prm", [128, 128], F32, kind="ExternalInput")
    p64 = nc.dram_tensor("p64", [128, 128], F32, kind="ExternalInput")
    y = nc.dram_tensor("y", [T, D], F32, kind="ExternalOutput")

    with tile.TileContext(nc) as tc:
        with (
            tc.tile_pool(name="persist", bufs=1) as pp,
            tc.tile_pool(name="work", bufs=2) as wk,
            tc.tile_pool(name="work1", bufs=1) as wk1,
            tc.tile_pool(name="att", bufs=2) as at,
            tc.tile_pool(name="ysb", bufs=1) as yp,
            tc.tile_pool(name="pmm", bufs=2, space="PSUM") as pmm,
            tc.tile_pool(name="pms", bufs=1, space="PSUM") as pms,
            tc.tile_pool(name="psc", bufs=1, space="PSUM") as psc,
            tc.tile_pool(name="pz", bufs=1, space="PSUM") as pz,
        ):
            # ---- persistent loads ----
            xts = pp.tile([128, KT, T], BF16, tag="xts")
            nc.gpsimd.dma_start(out=xts, in_=xT.rearrange("(k p) t -> p k t", p=128))
            wq = pp.tile([128, KT, J], BF16, tag="wq")
            nc.gpsimd.dma_start(out=wq, in_=wqT.rearrange("(k p) m -> p k m", p=128))
            wk_ = pp.tile([128, KT, J], BF16, tag="wk")
            nc.gpsimd.dma_start(out=wk_, in_=wkT.rearrange("(k p) m -> p k m", p=128))
            wv = pp.tile([128, KT, J], BF16, tag="wv")
            nc.gpsimd.dma_start(out=wv, in_=wvT.rearrange("(k p) m -> p k m", p=128))
            wp = pp.tile([128, D], BF16, tag="wp")
            nc.gpsimd.dma_start(out=wp, in_=wpT[:, :])
            csb = pp.tile([J, T], F32, tag="csb")
            nc.sync.dma_start(out=csb, in_=Ct[:, :])
            ssb = pp.tile([J, T], F32, tag="ssb")
            nc.sync.dma_start(out=ssb, in_=St[:, :])
            tri_sb = pp.tile([128, 128], F32, tag="tri")
            nc.sync.dma_start(out=tri_sb, in_=tri[:, :])
            o2r_sb = pp.tile([128, 128], BF16, tag="o2r")
            nc.gpsimd.dma_start(out=o2r_sb, in_=o2r[:, :])
            prm_sb = pp.tile([128, 128], BF16, tag="prm")
            nc.gpsimd.dma_start(out=prm_sb, in_=prm[:, :])
            p64_sb = pp.tile([128, 128], BF16, tag="p64")
            nc.gpsimd.dma_start(out=p64_sb, in_=p64[:, :])
            eps_sb = pp.tile([128, 1], F32, tag="eps")
            nc.vector.memset(eps_sb, EPS)
            lam_sb = pp.tile([128, 2], F32, tag="lam")
            _lap = lam.ap()
            lam_b = bass.AP(tensor=_lap.tensor, offset=_lap.offset,
                            ap=[[0, 128], [1, 2]])
            nc.sync.dma_start(out=lam_sb, in_=lam_b)

            # scale Wv by lambda0 once
            for kk in range(KT):
                nc.vector.tensor_scalar_mul(wv[:, kk, :], wv[:, kk, :],
                                            lam_sb[:, 0:1])

            # v_aug: [v_h0 | ones64 | ones64 | v_h1] per s-tile
            vaug = pp.tile([128, NT, 4, HD], BF16, tag="vaug")
            nc.gpsimd.memset(vaug[:, :, 1:3, :], 1.0)

            kh = pp.tile([J, T], BF16, tag="kh")

            def qk_rope(dst, wmat, tsl, tag):
                """QKV->transposed + rmsnorm + rope for one tensor/chunk."""
                q_ps = pmm.tile([128, 512], F32, tag="mm")
                for kk in range(KT):
                    nc.tensor.matmul(q_ps, wmat[:, kk, :], xts[:, kk, tsl],
                                     start=(kk == 0), stop=(kk == KT - 1))
                q2 = wk.tile([128, 512], BF16, tag="q2")
                nc.scalar.square(q2, q_ps)
                ms_ps = pms.tile([128, 512], F32, tag="ms")
                nc.tensor.matmul(ms_ps, o2r_sb, q2, start=True, stop=True)
                lnm = wk1.tile([128, 512], F32, tag="lnm")
                nc.scalar.activation(lnm, ms_ps, AF.Ln, bias=eps_sb, scale=1.0 / HD)
                rq = wk.tile([128, 512], F32, tag="rq")
                nc.scalar.activation(rq, lnm, AF.Exp, bias=0.0, scale=-0.5)
                qn = wk.tile([128, 512], BF16, tag="qn")
                nc.vector.tensor_tensor(qn, q_ps, rq, OP.mult)
                qs_ps = pmm.tile([128, 512], F32, tag="mm")
                nc.tensor.matmul(qs_ps, prm_sb, qn, start=True, stop=True)
                t1 = wk1.tile([128, 512], F32, tag="t1")
                nc.gpsimd.tensor_mul(t1, qn, csb[:, tsl])
                t2 = wk1.tile([128, 512], F32, tag="t2")
                nc.vector.tensor_tensor(t2, qs_ps, ssb[:, tsl], OP.mult)
                nc.gpsimd.tensor_add(dst, t1, t2)

            for tcn in range(NCH):
                tsl = slice(512 * tcn, 512 * (tcn + 1))

                # ---- stage B: q,k (transposed) + v (t-layout) ----
                qh = wk.tile([J, 512], BF16, tag="qh")
                qk_rope(qh, wq, tsl, "q")
                qk_rope(kh[:, tsl], wk_, tsl, "k")

                vic_c = wk.tile([128, 4, J], F32, tag="vic")
                nc.sync.dma_start(
                    out=vic_c,
                    in_=vic[tsl, :].rearrange("(ti p) c -> p ti c", p=128))
                for ti in range(4):
                    st = 4 * tcn + ti
                    v_ps = pmm.tile([128, 512], F32, tag="mm")
                    for kk in range(KT):
                        nc.tensor.matmul(
                            v_ps[:, 0:J],
                            xts[:, kk, 128 * st:128 * (st + 1)],
                            wv[:, kk, :],
                            start=(kk == 0), stop=(kk == KT - 1))
                    # vaug[:, st, {0,3}, :] = vic*lam1 + v_ps
                    out_ap = vaug[:, st, 0:4:3, :]
                    nc.vector.scalar_tensor_tensor(
                        out_ap, vic_c[:, ti, :].rearrange("p (h d) -> p h d", h=2),
                        lam_sb[:, 1:2],
                        v_ps[:, 0:J].rearrange("p (h d) -> p h d", h=2),
                        OP.mult, OP.add)

                # ---- stage C: attention for this chunk ----
                zt2 = pz.tile([128, 2, 512], F32, tag="zt2")
                n_st = 4 * (tcn + 1)
                for jst in range(n_st):
                    loc0 = max(0, 128 * jst - 512 * tcn)
                    nn = 512 - loc0
                    sc = psc.tile([128, 2, 512], F32, tag="sc")
                    for h in range(HS):
                        nc.tensor.matmul(
                            sc[:, h, loc0:],
                            kh[64 * h:64 * (h + 1), 128 * jst:128 * (jst + 1)],
                            qh[64 * h:64 * (h + 1), loc0:],
                            start=True, stop=True)
                    aT = at.tile([128, 2, 512], BF16, tag="aT")
                    if loc0 == 0:
                        nc.scalar.activation(aT, sc, AF.Exp, bias=0.0,
                                             scale=1.0 / 8.0)
                    else:
                        for h in range(HS):
                            nc.scalar.activation(aT[:, h, loc0:], sc[:, h, loc0:],
                                                 AF.Exp, bias=0.0, scale=1.0 / 8.0)
                    if jst >= 4 * tcn:  # diagonal s-tile: apply causal triangle
                        for h in range(HS):
                            nc.gpsimd.tensor_mul(aT[:, h, loc0:loc0 + 128],
                                                 aT[:, h, loc0:loc0 + 128], tri_sb)
                    # z matmuls: h0 lhsT=[v|ones] -> z rows 0:64, Zrep 64:128
                    #            h1 lhsT=[ones|v] -> Zrep 0:64, z rows 64:128
                    for h in range(HS):
                        nc.tensor.matmul(
                            zt2[:, h, loc0:],
                            vaug[:, jst, 2 * h:2 * h + 2, :],
                            aT[:, h, loc0:],
                            start=(jst == 0), stop=(jst == n_st - 1))

                # recipZ = exp(-ln(Z)); Zrep on rows 64:128 (h0) / 0:64 (h1).
                # Compute recip in-place on those lanes, then swap the two
                # 64-lane halves with a permutation matmul so recipZ lands on
                # the same lanes as each head's z rows.
                zw = at.tile([128, 2, 512], F32, tag="zw")
                nc.scalar.activation(zw[64:128, 0, :], zt2[64:128, 0, :], AF.Ln,
                                     bias=0.0, scale=1.0)
                nc.scalar.activation(zw[0:64, 1, :], zt2[0:64, 1, :], AF.Ln,
                                     bias=0.0, scale=1.0)
                rzb = at.tile([128, 512], BF16, tag="rzb")
                nc.scalar.activation(rzb[64:128, :], zw[64:128, 0, :], AF.Exp,
                                     bias=0.0, scale=-1.0)
                nc.scalar.activation(rzb[0:64, :], zw[0:64, 1, :], AF.Exp,
                                     bias=0.0, scale=-1.0)
                rzs_ps = pmm.tile([128, 512], F32, tag="mm")
                nc.tensor.matmul(rzs_ps, p64_sb, rzb, start=True, stop=True)
                rz = at.tile([128, 512], F32, tag="rz")
                nc.vector.tensor_copy(rz, rzs_ps)
                zt_all = wk.tile([128, 512], BF16, tag="zta")
                nc.vector.tensor_tensor(zt_all[0:64, :], zt2[0:64, 0, :],
                                        rz[0:64, :], OP.mult)
                nc.vector.tensor_tensor(zt_all[64:128, :], zt2[64:128, 1, :],
                                        rz[64:128, :], OP.mult)

                # ---- stage D: partial c_proj for this chunk ----
                y_sb = yp.tile([128, 4, D], F32, tag="ysb")
                for ti in range(4):
                    for oc in range(2):
                        y_ps = pmm.tile([128, 512], F32, tag="mm")
                        nc.tensor.matmul(y_ps,
                                         zt_all[:, 128 * ti:128 * (ti + 1)],
                                         wp[:, 512 * oc:512 * (oc + 1)],
                                         start=True, stop=True)
                        if (ti + oc) % 2 == 0:
                            nc.vector.tensor_copy(
                                y_sb[:, ti, 512 * oc:512 * (oc + 1)], y_ps)
                        else:
                            nc.scalar.copy(
                                y_sb[:, ti, 512 * oc:512 * (oc + 1)], y_ps)
                nc.sync.dma_start(
                    out=y[tsl, :].rearrange("(ti p) o -> p ti o", p=128),
                    in_=y_sb)

    nc.finalize()
    return nc


def _host_prep(x, vi, Wq, Wk, Wv, Wproj, lambdas):
    x = np.asarray(x, np.float32)[0]
    vi = np.asarray(vi, np.float32)[0]
    Wq, Wk, Wv = (np.asarray(a, np.float32) for a in (Wq, Wk, Wv))
    Wp = np.asarray(Wproj, np.float32)
    lam = np.asarray(lambdas, np.float32)

    xT = np.ascontiguousarray(x.T)
    quarter = HD // 4
    inv_freq = (1.0 / 1024.0) ** np.linspace(0.0, 1.0, quarter, dtype=np.float32)
    inv_freq = np.concatenate([inv_freq, np.zeros(quarter, np.float32)])
    th = np.arange(T, dtype=np.float32)[:, None] * inv_freq[None, :]
    cos, sin = np.cos(th).astype(np.float32), np.sin(th).astype(np.float32)
    C = np.zeros((J, T), np.float32)
    S = np.zeros((J, T), np.float32)
    for h in range(HS):
        C[h * 64:h * 64 + 32] = cos.T[:32]
        C[h * 64 + 32:h * 64 + 64] = cos.T[:32]
        S[h * 64:h * 64 + 32] = sin.T[:32]
        S[h * 64 + 32:h * 64 + 64] = -sin.T[:32]
    tri = np.triu(np.ones((128, 128), np.float32))
    o2r = np.zeros((128, 128), np.float32)
    o2r[0:64, 0:64] = 1.0
    o2r[64:128, 64:128] = 1.0
    prm = np.zeros((128, 128), np.float32)
    for i in range(128):
        src = i + 32 if (i % 64) < 32 else i - 32
        prm[src, i] = 1.0
    p64 = np.zeros((128, 128), np.float32)
    for i in range(128):
        p64[(i + 64) % 128, i] = 1.0

    in_maps = []
    for c in range(N_CORES):
        rows = slice(J * c, J * (c + 1))
        in_maps.append({
            "xT": xT,
            "wqT": np.ascontiguousarray(Wq[rows, :].T),
            "wkT": np.ascontiguousarray(Wk[rows, :].T),
            "wvT": np.ascontiguousarray(Wv[rows, :].T),
            "wpT": np.ascontiguousarray(Wp[:, rows].T),
            "vic": np.ascontiguousarray(vi[:, rows]),
            "lam": lam, "Ct": C, "St": S,
            "tri": tri, "o2r": o2r, "prm": prm, "p64": p64,
        })
    return in_maps


_NC = None


def kernel(x, vi, Wq, Wk, Wv, Wproj, lambdas):
    global _NC
    if _NC is None:
        _NC = build_nc()
    in_maps = _host_prep(x, vi, Wq, Wk, Wv, Wproj, lambdas)
    trace = bool(int(os.environ.get("KERNEL_TRACE", "0")))
    res = run_bass_kernel_spmd(_NC, in_maps, core_ids=list(range(N_CORES)),
                               trace=trace)
    if trace and res.exec_time_ns is not None:
        print(f"HW exec time: {res.exec_time_ns} ns")
    out = np.zeros((T, D), np.float32)
    for c in range(N_CORES):
        out += res.results[c]["y"]
    return out.reshape(1, T, D)



# revision 4
# speedup vs baseline: 2.0095x; 2.0095x over previous
"""Head-sharded causal self-attention (value-residual + RMSNorm + RoPE) for 8 TRN2 cores.

Sharding: 2 heads per core (tensor parallel). Each core computes q/k/v for its
128 dims, full causal attention for its heads, and a partial c_proj output;
the host sums the 8 partial [T, D] outputs (the TP all-reduce).

v2 restructure vs baseline:
  - Two-phase schedule so the scalar (ACT) engine needs exactly 2 table
    loads: phase A does all Square+Ln (natural_log table) for the RMSNorm
    stats of all chunks; everything after runs out of the exp table.
  - lambda-mix of vi folded into an augmented V GEMM (x|vi contraction).
  - causal mask applied additively in PSUM via a -240*I x tril matmul
    (no masked multiply on gpsimd after exp).
  - softmax denominator via DVE reciprocal_approx_fast (no Ln/Exp).
  - bf16 DRAM inputs + bf16 partial-y output (halves HBM traffic).
  - attention inner loop software-pipelined (QK(j+1) issued before PV(j));
    c_proj matmuls of chunk c-1 drained into tensor-idle slots of chunk c.
"""
import os
import sys

sys.path.insert(0, "/opt/trn_rl_repo")

import numpy as np
import ml_dtypes

import concourse.bacc as bacc
import concourse.tile as tile
import concourse.bass as bass
from concourse import mybir
from concourse.bass_utils import run_bass_kernel_spmd

N_CORES = 8
T, D, H, HD = 2048, 1024, 16, 64
HS = H // N_CORES            # 2 heads per core
J = HS * HD                  # 128
NT = T // 128                # 16 s-tiles
NCH = T // 512               # 4 chunks
KT = D // 128                # 8 contraction tiles for q/k
KV = KT + 1                  # 9 for the augmented v GEMM (x | vi)
F32 = mybir.dt.float32
BF16 = mybir.dt.bfloat16
AF = mybir.ActivationFunctionType
OP = mybir.AluOpType
EPS = float(np.finfo(np.float32).eps)
BF = ml_dtypes.bfloat16


def build_nc():
    nc = bacc.Bacc("TRN2", target_bir_lowering=False, debug=False,
                   num_devices=N_CORES)

    xaT = nc.dram_tensor("xaT", [KV * 128, T], BF16, kind="ExternalInput")
    wqT = nc.dram_tensor("wqT", [D, J], BF16, kind="ExternalInput")
    wkT = nc.dram_tensor("wkT", [D, J], BF16, kind="ExternalInput")
    wvT = nc.dram_tensor("wvT", [KV * 128, J], BF16, kind="ExternalInput")
    wpT = nc.dram_tensor("wpT", [J, D], BF16, kind="ExternalInput")
    Ct = nc.dram_tensor("Ct", [J, T], BF16, kind="ExternalInput")
    St = nc.dram_tensor("St", [J, T], BF16, kind="ExternalInput")
    o2r = nc.dram_tensor("o2r", [128, 128], BF16, kind="ExternalInput")
    prm = nc.dram_tensor("prm", [128, 128], BF16, kind="ExternalInput")
    p64 = nc.dram_tensor("p64", [128, 128], BF16, kind="ExternalInput")
    mI = nc.dram_tensor("mI", [128, 128], BF16, kind="ExternalInput")
    tlo = nc.dram_tensor("tlo", [128, 128], BF16, kind="ExternalInput")
    y = nc.dram_tensor("y", [T, D], BF16, kind="ExternalOutput")

    with tile.TileContext(nc) as tc:
        with tc.tile_pool(name="persist", bufs=1) as pp:
            # ---- persistent loads ----
            xa = pp.tile([128, KV, T], BF16, tag="xa")
            for c in range(NCH):
                tsl = slice(512 * c, 512 * (c + 1))
                nc.gpsimd.dma_start(
                    out=xa[:, :, tsl],
                    in_=xaT[:, tsl].rearrange("(k p) t -> p k t", p=128))
            wq = pp.tile([128, KT, J], BF16, tag="wq")
            nc.sync.dma_start(out=wq, in_=wqT.rearrange("(k p) m -> p k m", p=128))
            wk_ = pp.tile([128, KT, J], BF16, tag="wk")
            nc.sync.dma_start(out=wk_, in_=wkT.rearrange("(k p) m -> p k m", p=128))
            wv = pp.tile([128, KV, J], BF16, tag="wv")
            nc.sync.dma_start(out=wv, in_=wvT.rearrange("(k p) m -> p k m", p=128))
            wp = pp.tile([128, D], BF16, tag="wp")
            nc.sync.dma_start(out=wp, in_=wpT[:, :])
            csb = pp.tile([J, T], BF16, tag="csb")
            nc.sync.dma_start(out=csb, in_=Ct[:, :])
            ssb = pp.tile([J, T], BF16, tag="ssb")
            nc.sync.dma_start(out=ssb, in_=St[:, :])
            o2r_sb = pp.tile([128, 128], BF16, tag="o2r")
            nc.sync.dma_start(out=o2r_sb, in_=o2r[:, :])
            prm_sb = pp.tile([128, 128], BF16, tag="prm")
            nc.sync.dma_start(out=prm_sb, in_=prm[:, :])
            p64_sb = pp.tile([128, 128], BF16, tag="p64")
            nc.sync.dma_start(out=p64_sb, in_=p64[:, :])
            mI_sb = pp.tile([128, 128], BF16, tag="mI")
            nc.sync.dma_start(out=mI_sb, in_=mI[:, :])
            tlo_sb = pp.tile([128, 128], BF16, tag="tlo")
            nc.sync.dma_start(out=tlo_sb, in_=tlo[:, :])
            eps_sb = pp.tile([128, 1], F32, tag="eps")
            nc.vector.memset(eps_sb, EPS)

            # v_aug: [v_h0 | ones64 | ones64 | v_h1] per s-tile
            vaug = pp.tile([128, NT, 4, HD], BF16, tag="vaug")
            nc.gpsimd.memset(vaug[:, :, 1:3, :], 1.0)

            qh = pp.tile([J, T], BF16, tag="qh")    # roped raw q
            kh = pp.tile([J, T], BF16, tag="kh")    # roped raw k
            lnq = pp.tile([128, T], F32, tag="lnq")  # ln(mean-square + eps)
            lnk = pp.tile([128, T], F32, tag="lnk")
            rq = pp.tile([128, T], BF16, tag="rq")   # rsqrt factors
            rk = pp.tile([128, T], BF16, tag="rk")
            qhn = pp.tile([J, T], BF16, tag="qhn")  # normalized roped q
            khn = pp.tile([J, T], BF16, tag="khn")

            # ================= phase A =================
            # q/k/v GEMMs, squares + Ln stats, raw rope. Scalar engine uses
            # only SQUARE/LN here -> stays in the natural_log table.
            with (
                tc.tile_pool(name="pq", bufs=1, space="PSUM") as pq,
                tc.tile_pool(name="pk", bufs=1, space="PSUM") as pk,
                tc.tile_pool(name="pv", bufs=2, space="PSUM") as pv,
                tc.tile_pool(name="pms", bufs=2, space="PSUM") as pms,
                tc.tile_pool(name="pr", bufs=2, space="PSUM") as pr,
                tc.tile_pool(name="aw", bufs=2) as aw,
            ):
                for c in range(NCH):
                    tsl = slice(512 * c, 512 * (c + 1))
                    q_ps = pq.tile([128, 512], F32, tag="q")
                    for kk in range(KT):
                        nc.tensor.matmul(q_ps, wq[:, kk, :], xa[:, kk, tsl],
                                         start=(kk == 0), stop=(kk == KT - 1))
                    k_ps = pk.tile([128, 512], F32, tag="k")
                    for kk in range(KT):
                        nc.tensor.matmul(k_ps, wk_[:, kk, :], xa[:, kk, tsl],
                                         start=(kk == 0), stop=(kk == KT - 1))
                    # squares on scalar; raw copies on vector
                    q2 = aw.tile([128, 512], BF16, tag="q2")
                    nc.scalar.square(q2, q_ps)
                    qraw = aw.tile([128, 512], BF16, tag="qraw")
                    nc.vector.tensor_copy(qraw, q_ps)
                    k2 = aw.tile([128, 512], BF16, tag="k2")
                    nc.scalar.square(k2, k_ps)
                    kraw = aw.tile([128, 512], BF16, tag="kraw")
                    nc.vector.tensor_copy(kraw, k_ps)
                    # v (augmented with lambda1*vi identity rows)
                    for ti in range(4):
                        st = 4 * c + ti
                        v_ps = pv.tile([128, 128], F32, tag="v")
                        for kk in range(KV):
                            nc.tensor.matmul(
                                v_ps,
                                xa[:, kk, 128 * st:128 * (st + 1)],
                                wv[:, kk, :],
                                start=(kk == 0), stop=(kk == KV - 1))
                        nc.vector.tensor_copy(
                            vaug[:, st, 0:4:3, :],
                            v_ps.rearrange("p (h d) -> p h d", h=2))
                    # mean-square stats -> Ln
                    msq = pms.tile([128, 512], F32, tag="ms")
                    nc.tensor.matmul(msq, o2r_sb, q2, start=True, stop=True)
                    nc.scalar.activation(lnq[:, tsl], msq, AF.Ln,
                                         bias=eps_sb, scale=1.0 / HD)
                    msk = pms.tile([128, 512], F32, tag="ms")
                    nc.tensor.matmul(msk, o2r_sb, k2, start=True, stop=True)
                    nc.scalar.activation(lnk[:, tsl], msk, AF.Ln,
                                         bias=eps_sb, scale=1.0 / HD)
                    # raw rope: qh = qraw*C + (P qraw)*S  (gpsimd + vector)
                    qs_ps = pr.tile([128, 512], F32, tag="rot")
                    nc.tensor.matmul(qs_ps, prm_sb, qraw, start=True, stop=True)
                    t1 = aw.tile([128, 512], BF16, tag="t1")
                    nc.gpsimd.tensor_mul(t1, qraw, csb[:, tsl])
                    t2 = aw.tile([128, 512], BF16, tag="t2")
                    nc.vector.tensor_tensor(t2, qs_ps, ssb[:, tsl], OP.mult)
                    nc.gpsimd.tensor_add(qh[:, tsl], t1, t2)
                    ks_ps = pr.tile([128, 512], F32, tag="rot")
                    nc.tensor.matmul(ks_ps, prm_sb, kraw, start=True, stop=True)
                    t3 = aw.tile([128, 512], BF16, tag="t3")
                    nc.gpsimd.tensor_mul(t3, kraw, csb[:, tsl])
                    t4 = aw.tile([128, 512], BF16, tag="t4")
                    nc.vector.tensor_tensor(t4, ks_ps, ssb[:, tsl], OP.mult)
                    nc.gpsimd.tensor_add(kh[:, tsl], t3, t4)

            # ================= phase C =================
            # per-chunk: normalize prologue (exp table) + attention + c_proj.
            with (
                tc.tile_pool(name="psc", bufs=4, space="PSUM") as psc,
                tc.tile_pool(name="pz", bufs=1, space="PSUM") as pz,
                tc.tile_pool(name="py", bufs=2, space="PSUM") as py,
                tc.tile_pool(name="at", bufs=3) as at,
                tc.tile_pool(name="zw", bufs=2) as zw,
                tc.tile_pool(name="yo", bufs=2) as yo,
            ):
                deferred = []

                def drain(n):
                    for _ in range(min(n, len(deferred))):
                        deferred.pop(0)()

                for c in range(NCH):
                    tsl = slice(512 * c, 512 * (c + 1))
                    n_st = 4 * (c + 1)

                    # prologue: rsqrt factors + normalized q/k for this chunk
                    nc.scalar.activation(rq[:, tsl], lnq[:, tsl], AF.Exp,
                                         bias=0.0, scale=-0.5)
                    nc.scalar.activation(rk[:, tsl], lnk[:, tsl], AF.Exp,
                                         bias=0.0, scale=-0.5)
                    nc.vector.tensor_tensor(qhn[:, tsl], qh[:, tsl],
                                            rq[:, tsl], OP.mult)
                    nc.vector.tensor_tensor(khn[:, tsl], kh[:, tsl],
                                            rk[:, tsl], OP.mult)

                    zt2 = pz.tile([128, HS, 512], F32, tag="zt")
                    sc_t = {}
                    aT_t = {}

                    def emit_qk(j, c=c, sc_t=sc_t):
                        loc0 = max(0, 128 * j - 512 * c)
                        is_diag = 128 * j >= 512 * c
                        sc_t[j] = []
                        for h in range(HS):
                            sch = psc.tile([128, 512], F32, tag="sc")
                            nc.tensor.matmul(
                                sch[:, loc0:],
                                khn[64 * h:64 * (h + 1),
                                    128 * j:128 * (j + 1)],
                                qhn[64 * h:64 * (h + 1),
                                    512 * c + loc0:512 * (c + 1)],
                                start=True, stop=not is_diag)
                            if is_diag:
                                # diagonal s-tile: add -240 upper-tri mask
                                nc.tensor.matmul(
                                    sch[:, loc0:loc0 + 128], mI_sb, tlo_sb,
                                    start=False, stop=True)
                            sc_t[j].append(sch)

                    def emit_exp(j, c=c, sc_t=sc_t, aT_t=aT_t):
                        loc0 = max(0, 128 * j - 512 * c)
                        aT = at.tile([128, HS, 512], BF16, tag="aT")
                        if loc0 == 0:
                            for h in range(HS):
                                nc.scalar.activation(aT[:, h, :], sc_t[j][h],
                                                     AF.Exp, bias=0.0,
                                                     scale=1.0 / 8.0)
                        else:
                            for h in range(HS):
                                nc.scalar.activation(
                                    aT[:, h, loc0:], sc_t[j][h][:, loc0:],
                                    AF.Exp, bias=0.0, scale=1.0 / 8.0)
                        del sc_t[j]
                        aT_t[j] = aT

                    def emit_pv(j, c=c, n_st=n_st, zt2=zt2, aT_t=aT_t):
                        loc0 = max(0, 128 * j - 512 * c)
                        aT = aT_t.pop(j)
                        for h in range(HS):
                            nc.tensor.matmul(
                                zt2[:, h, loc0:],
                                vaug[:, j, 2 * h:2 * h + 2, :],
                                aT[:, h, loc0:],
                                start=(j == 0), stop=(j == n_st - 1))

                    # software-pipelined attention loop
                    emit_qk(0)
                    for j in range(n_st):
                        emit_exp(j)
                        if j + 1 < n_st:
                            emit_qk(j + 1)
                        drain(3)
                        emit_pv(j)

                    # epilogue: softmax denominator + c_proj (deferred so its
                    # tensor work fills gaps of the next chunk's loop)
                    zzb = zw.tile([128, 512], BF16, tag="zzb")
                    nc.vector.tensor_copy(zzb[0:64, :], zt2[0:64, 1, :])
                    nc.vector.tensor_copy(zzb[64:128, :], zt2[64:128, 0, :])
                    ysb = yo.tile([128, 4, D], BF16, tag="ysb")

                    def emit_zn(c=c, zt2=zt2, zzb=zzb):
                        zsw_ps = py.tile([128, 512], F32, tag="y")
                        nc.tensor.matmul(zsw_ps, p64_sb, zzb,
                                         start=True, stop=True)
                        zsw = zw.tile([128, 512], F32, tag="zsw")
                        nc.vector.tensor_copy(zsw, zsw_ps)
                        rzf = zw.tile([128, 512], F32, tag="rzf")
                        nc.vector.reciprocal_approx_fast(out=rzf, in_=zsw)
                        ztn = zw.tile([128, 512], BF16, tag="ztn")
                        nc.vector.tensor_tensor(ztn[0:64, :], zt2[0:64, 0, :],
                                                rzf[0:64, :], OP.mult)
                        nc.vector.tensor_tensor(ztn[64:128, :],
                                                zt2[64:128, 1, :],
                                                rzf[64:128, :], OP.mult)
                        return ztn

                    zn_box = {}

                    def zn_thunk(zn_box=zn_box, fn=emit_zn):
                        zn_box["ztn"] = fn()

                    deferred.append(zn_thunk)

                    def cproj_thunk(ti, oc, c=c, ysb=ysb, zn_box=zn_box):
                        def go():
                            ztn = zn_box["ztn"]
                            y_ps = py.tile([128, 512], F32, tag="y")
                            nc.tensor.matmul(
                                y_ps,
                                ztn[:, 128 * ti:128 * (ti + 1)],
                                wp[:, 512 * oc:512 * (oc + 1)],
                                start=True, stop=True)
                            nc.vector.tensor_copy(
                                ysb[:, ti, 512 * oc:512 * (oc + 1)], y_ps)
                        return go

                    for ti in range(4):
                        for oc in range(2):
                            deferred.append(cproj_thunk(ti, oc))

                    def dma_thunk(c=c, ysb=ysb, tsl=tsl):
                        nc.sync.dma_start(
                            out=y[tsl, :].rearrange("(ti p) o -> p ti o",
                                                    p=128),
                            in_=ysb)

                    deferred.append(dma_thunk)

                drain(len(deferred))

    nc.finalize()
    return nc


def _host_prep(x, vi, Wq, Wk, Wv, Wproj, lambdas):
    x = np.asarray(x, np.float32)[0]
    vi = np.asarray(vi, np.float32)[0]
    Wq, Wk, Wv = (np.asarray(a, np.float32) for a in (Wq, Wk, Wv))
    Wp = np.asarray(Wproj, np.float32)
    lam = np.asarray(lambdas, np.float32)

    xT = np.ascontiguousarray(x.T)
    quarter = HD // 4
    inv_freq = (1.0 / 1024.0) ** np.linspace(0.0, 1.0, quarter, dtype=np.float32)
    inv_freq = np.concatenate([inv_freq, np.zeros(quarter, np.float32)])
    th = np.arange(T, dtype=np.float32)[:, None] * inv_freq[None, :]
    cos, sin = np.cos(th).astype(np.float32), np.sin(th).astype(np.float32)
    C = np.zeros((J, T), np.float32)
    S = np.zeros((J, T), np.float32)
    for h in range(HS):
        C[h * 64:h * 64 + 32] = cos.T[:32]
        C[h * 64 + 32:h * 64 + 64] = cos.T[:32]
        S[h * 64:h * 64 + 32] = sin.T[:32]
        S[h * 64 + 32:h * 64 + 64] = -sin.T[:32]
    o2r = np.zeros((128, 128), np.float32)
    o2r[0:64, 0:64] = 1.0
    o2r[64:128, 64:128] = 1.0
    prm = np.zeros((128, 128), np.float32)
    for i in range(128):
        src = i + 32 if (i % 64) < 32 else i - 32
        prm[src, i] = 1.0
    p64 = np.zeros((128, 128), np.float32)
    for i in range(128):
        p64[(i + 64) % 128, i] = 1.0
    mI = -240.0 * np.eye(128, dtype=np.float32)
    tlo = np.tril(np.ones((128, 128), np.float32), -1)

    bf = lambda a: np.ascontiguousarray(a).astype(BF)

    in_maps = []
    for c in range(N_CORES):
        rows = slice(J * c, J * (c + 1))
        xaT = np.concatenate([xT, (lam[1] * vi[:, rows]).T], axis=0)
        wvT = np.concatenate([(lam[0] * Wv[rows, :]).T,
                              np.eye(J, dtype=np.float32)], axis=0)
        in_maps.append({
            "xaT": bf(xaT),
            "wqT": bf(Wq[rows, :].T),
            "wkT": bf(Wk[rows, :].T),
            "wvT": bf(wvT),
            "wpT": bf(Wp[:, rows].T),
            "Ct": bf(C), "St": bf(S),
            "o2r": bf(o2r), "prm": bf(prm), "p64": bf(p64),
            "mI": bf(mI), "tlo": bf(tlo),
        })
    return in_maps


_NC = None


def kernel(x, vi, Wq, Wk, Wv, Wproj, lambdas):
    global _NC
    if _NC is None:
        _NC = build_nc()
    in_maps = _host_prep(x, vi, Wq, Wk, Wv, Wproj, lambdas)
    trace = bool(int(os.environ.get("KERNEL_TRACE", "0")))
    res = run_bass_kernel_spmd(_NC, in_maps, core_ids=list(range(N_CORES)),
                               trace=trace)
    if trace and res.exec_time_ns is not None:
        print(f"HW exec time: {res.exec_time_ns} ns")
    out = np.zeros((T, D), np.float32)
    for c in range(N_CORES):
        out += res.results[c]["y"].astype(np.float32)
    return out.reshape(1, T, D)
